# revision 76
# baseline (speedup 1.0000x reference)
"""Trainium2 Bass kernel for the GNN message-passing module.

Per-sample pipeline (data-parallel: one batch element per NeuronCore):
  1. pass 1: segment sums via one-hot matmul on PE. The pixel-major
     transposed x (fp8 e4m3) and the pixel-major one-hot (fp8, exact)
     are prepared on the host, so pass 1 is a pure 128-matmul fp8
     accumulation chain (two alternating PSUM banks) -- no on-chip
     transposes, no PSUM evacuation. Segment counts -> 1/count come
     from the host (pure index preprocessing).
  2. small "middle" stage: means, M=W@W^T, Mahalanobis adjacency folded
     into a (K, C_out) table: table2T = adj-weighted conv'd means,
  3. pass 2: out = conv_w @ x + table2T[index] via PE matmuls over
     2048-px supergroups (each weight load serves four 512-col
     matmuls); the gather is an fp8 one-hot matmul accumulated into the
     same PSUM as the 1x1 conv. PSUM is evacuated by ScalarE only
     (DVE reads PSUM ~4x slower). Output is written bf16 and upcast on
     the host.

Scheduling notes (measured on HW):
  - DMA transfers complete in rough emission order across a shared
    ~16-queue fabric at ~190-245 GB/s effective: emission order is the
    only priority control, so pass-1 feeds go first, small params are
    interleaved after the third group, pass-2 data follows.
  - The PE clock gate (HAM) defaults to half rate and re-demotes on low
    duty cycle; dep-free warm-up/filler matmuls bracket the DMA-paced
    stretches.
  - A post-pass strips per-instruction semaphore increments whose
    values are never wait targets (saves ~26ns/instruction on PE).

Math notes:
  adj[i,j] = exp(-(m_j-m_i)^T M (m_j-m_i)) with zero diagonal, M=W W^T.
  Using G = means @ M @ means^T, g = diag(G):
    adj[i,j] = exp(2G_ij - g_i - g_j) - delta_ij
  agg = adj @ means  =>  out += conv_w @ agg[index]
  table2T[k,:] = e^{-g_k} * (aggT_raw^T @ conv_w^T)[k,:] - (means @ conv_w^T)[k,:]
  where aggT_raw[:,i] = sum_j B[j,i] * (e^{-g_j} means[j,:]),
        B[i,j] = exp(2G_ij - g_i).

Precision: segment sums run on fp8 e4m3 pixels (segment means average
~256 pixels, so the 3% per-element quantization washes out); the conv
runs bf16 with fp32 PSUM accumulation; W/M/Q and the table matmuls run
bf16; the gather table is fp8 e4m3 (|table| ~ 0.5 << 240). Output is
bf16. Measured max rel err 9.6e-3 vs the 2e-2 gate.
"""

import os
import sys

import numpy as np


def _ensure_path():
    try:
        import concourse  # noqa: F401
    except ImportError:
        for p in ("/opt/trn_rl_repo", os.path.expanduser("~/.axon_site/_ro/trn_rl_repo")):
            if os.path.isdir(p) and p not in sys.path:
                sys.path.insert(0, p)


_ensure_path()
# persistent jax/XLA executable cache: makes repeat compiles of the same
# kernel cheap across processes (first compile of a variant is ~minutes).
os.environ.setdefault("JAX_COMPILATION_CACHE_DIR", "/tmp/jax_neff_cache")
os.environ.setdefault("JAX_PERSISTENT_CACHE_MIN_COMPILE_TIME_SECS", "10")

import concourse.bass as bass  # noqa: E402
import concourse.tile as tile  # noqa: E402
from concourse import bacc  # noqa: E402
from concourse import mybir  # noqa: E402
from concourse.masks import make_identity  # noqa: E402

F32 = mybir.dt.float32

# --- workaround: this walrus build rejects instructions carrying >2 sem
# waits ("Too many sync wait commands" in setupSyncWait). TileContext's exit
# drain accumulates one wait per outstanding processor (DMA queues etc.), so
# split them across NOPs emitted just before the drain. Semaphores are
# monotonic, so waiting earlier on the same conditions is equivalent.
_MAX_WAITS = 1
_drain_patched = False


def _patch_tile_drain():
    global _drain_patched
    if _drain_patched:
        return
    _drain_patched = True
    from concourse.vector_clock import ScopedClock

    orig = tile.TileContext._drain_and_barrier

    def patched(self, tick_clock, wait_clock):
        nc = self.nc
        probe = nc.sync.nop()
        wait_clock.add_sem_waits(
            probe.ins, ScopedClock({None: tick_clock.global_clock})
        )
        waits = list(probe.ins.sync_info.on_wait or [])
        chunks = [waits[i:i + _MAX_WAITS] for i in range(0, len(waits), _MAX_WAITS)]
        probe.ins.sync_info.on_wait = chunks[0] if chunks else []
        for chunk in chunks[1:]:
            nop = nc.sync.nop()
            nop.ins.sync_info = mybir.SyncInfo(on_wait=chunk, on_update=[])
        orig(self, tick_clock, wait_clock)
        _trim_redundant_waits(nc)
        _strip_unwaited_incs(nc)

    tile.TileContext._drain_and_barrier = patched


def _strip_unwaited_incs(nc):
    """Drop sem increments whose cumulative value is never a wait target,
    renumbering the surviving waits. Tile ticks a per-engine sem on every
    instruction, but consumers only wait on a handful of those values; the
    EVT_SEM register write costs ~26ns on the issuing engine. For each wait
    (sem, v) we keep the inc on the instruction whose cumulative count
    reaches v (so the wait still releases exactly when that instruction
    retires) and delete the rest.
    """
    import bisect

    # fn0's blocks execute linearly (entry -> body -> drain), so analyze them
    # as one concatenated sequence: the drain block's final-value waits then
    # renumber consistently with the body's stripped counts.
    if True:
        insts = [ins for blk in nc.m.functions[0].blocks
                 for ins in blk.instructions]
        nonmono = set()
        reset_at = {}  # sem -> first inst idx that resets it
        for idx, ins in enumerate(insts):
            si = ins.sync_info
            if si and si.on_update:
                for u in si.on_update:
                    if u.update_mode != "sem-inc" or not u.update_value:
                        nonmono.add(u.id)
            try:
                if ins.is_reset_sema:
                    lo, hi = ins.reset_range_start, ins.reset_range_stop
                    if lo is not None and hi is not None:
                        for s in range(lo, hi + 1):
                            reset_at.setdefault(s, idx)
            except Exception:
                pass
        # a reset only breaks renumbering if any update/wait of the sem comes
        # after it; the end-of-kernel cleanup reset does not.
        for idx, ins in enumerate(insts):
            si = ins.sync_info
            if not si:
                continue
            for u in (si.on_update or []):
                if u.id in reset_at and idx > reset_at[u.id]:
                    nonmono.add(u.id)
            for w in (si.on_wait or []):
                if w.id in reset_at and idx > reset_at[w.id]:
                    nonmono.add(w.id)
        # cumulative update list per sem: (cum_value, inst_idx, upd_pos, val)
        # NOTE: sync_info accessors materialize fresh objects per access, so
        # everything is tracked by (instruction index, position) and written
        # back with whole-list assignment.
        upd = {}
        cum = {}
        for idx, ins in enumerate(insts):
            si = ins.sync_info
            if not (si and si.on_update):
                continue
            for pos_u, u in enumerate(si.on_update):
                if u.update_mode != "sem-inc" or u.id in nonmono:
                    continue
                c = cum.get(u.id, 0) + u.update_value
                cum[u.id] = c
                upd.setdefault(u.id, []).append((c, idx, pos_u, u.update_value))
        # find wait targets
        keep = set()       # (sem, inst_idx, upd_pos)
        wait_fix = []      # (inst_idx, wait_pos, sem, target (idx, upd_pos))
        ok = True
        for idx, ins in enumerate(insts):
            si = ins.sync_info
            if not (si and si.on_wait):
                continue
            for pos_w, w in enumerate(si.on_wait):
                if w.wait_mode != "sem-ge-imm" or w.id in nonmono:
                    continue
                lst = upd.get(w.id)
                if not lst:
                    continue
                pos = bisect.bisect_left([c for c, _, _, _ in lst],
                                         w.wait_value)
                if pos == len(lst):
                    ok = False  # waits beyond final count: bail out
                    break
                _, tidx, tpos, _ = lst[pos]
                keep.add((w.id, tidx, tpos))
                wait_fix.append((idx, pos_w, w.id, (tidx, tpos)))
            if not ok:
                break
        if not ok:
            return
        # always keep the final inc of each sem (end-of-kernel accounting)
        for sem, lst in upd.items():
            _, tidx, tpos, _ = lst[-1]
            keep.add((sem, tidx, tpos))
        # renumber: new cumulative value per kept inc
        new_cum_at = {}  # (sem, inst_idx, upd_pos) -> new value
        for sem, lst in upd.items():
            nv = 0
            for c, idx, pos_u, val in lst:
                if (sem, idx, pos_u) in keep:
                    nv += val
                    new_cum_at[(sem, idx, pos_u)] = nv
        # rewrite waits (whole-list read-modify-write per instruction)
        import collections as _c
        fixes_by_inst = _c.defaultdict(list)
        for idx, pos_w, sem, (tidx, tpos) in wait_fix:
            fixes_by_inst[idx].append((pos_w, new_cum_at[(sem, tidx, tpos)]))
        for idx, fixes in fixes_by_inst.items():
            si = insts[idx].sync_info
            ow = list(si.on_wait)
            for pos_w, nv in fixes:
                ow[pos_w].wait_value = nv
            si.on_wait = ow
        # strip dropped updates
        drop_by_inst = _c.defaultdict(set)
        for sem, lst in upd.items():
            for c, idx, pos_u, val in lst:
                if (sem, idx, pos_u) not in keep:
                    drop_by_inst[idx].add(pos_u)
        for idx, drops in drop_by_inst.items():
            si = insts[idx].sync_info
            si.on_update = [u for pos_u, u in enumerate(si.on_update)
                            if pos_u not in drops]


def _trim_redundant_waits(nc):
    """Transitive wait reduction. Tile's add_semaphores is per-instruction
    minimal but not transitively minimal across processors: an instruction
    often carries waits already implied by (a) an earlier wait on the same
    engine, or (b) the closure of another wait it carries (the producer's own
    waits + in-order retirement on the producer's engine). This walrus build
    rejects instructions with >2 sync waits, so prune implied waits.

    Soundness assumptions: sem updates fire at instruction retirement;
    retirement is in-order per compute engine and per DMA queue sem (one sem
    per queue, FIFO); a kept wait on sem S>=v implies the v-reaching update's
    instruction retired, hence its dispatch-time holds and (non-DMA) all
    earlier same-engine updates.
    """
    import bisect

    for blk in nc.m.functions[0].blocks:
        insts = list(blk.instructions)
        n = len(insts)
        # sems that are ever decremented/reset are not monotonic; leave all
        # waits on them untouched and exclude them from closures (barrier
        # gather/release sems, end-of-kernel sem clears).
        nonmono = set()
        for ins in insts:
            si = ins.sync_info
            if si and si.on_update:
                for u in si.on_update:
                    if u.update_mode != "sem-inc":
                        nonmono.add(u.id)
            try:
                if ins.is_reset_sema:
                    lo = ins.reset_range_start
                    hi = ins.reset_range_stop
                    if lo is not None and hi is not None:
                        nonmono.update(range(lo, hi + 1))
            except Exception:
                pass
        upd = {}
        cum = {}
        own_cum_after = [None] * n
        eng_of = [str(i.engine) for i in insts]
        is_dma = [type(i).__name__ == "InstDMACopy" for i in insts]
        for idx, ins in enumerate(insts):
            si = ins.sync_info
            d = {}
            if si and si.on_update:
                for u in si.on_update:
                    if (u.update_mode != "sem-inc" or not u.update_value
                            or u.id in nonmono):
                        continue
                    c = cum.get(u.id, 0) + u.update_value
                    cum[u.id] = c
                    upd.setdefault(u.id, []).append((c, idx))
                    d[u.id] = c
            own_cum_after[idx] = d
        eng_cum_after = [None] * n
        run = {}
        for idx in range(n):
            e = eng_of[idx]
            m = dict(run.get(e, {}))
            if not is_dma[idx]:
                for s, c in own_cum_after[idx].items():
                    m[s] = c
            run[e] = m
            eng_cum_after[idx] = m

        def updater_idx(sem, v):
            lst = upd.get(sem)
            if not lst:
                return None
            pos = bisect.bisect_left(lst, (v, -1))
            if pos == len(lst):
                return None
            return lst[pos][1]

        holds_at = [None] * n
        last_eng = {}
        memo = {}

        def completion_holds(uidx):
            if uidx in memo:
                return memo[uidx]
            h = dict(holds_at[uidx] or {})
            src_cum = own_cum_after[uidx] if is_dma[uidx] else eng_cum_after[uidx]
            for s, c in src_cum.items():
                if h.get(s, 0) < c:
                    h[s] = c
            memo[uidx] = h
            return h

        n_dropped = 0
        for idx, ins in enumerate(insts):
            e = eng_of[idx]
            base = dict(holds_at[last_eng[e]]) if e in last_eng else {}
            si = ins.sync_info
            if si and si.on_wait:
                kept = []
                for w in si.on_wait:
                    if w.wait_mode != "sem-ge-imm" or w.id in nonmono:
                        kept.append(w)
                        continue
                    if base.get(w.id, 0) >= w.wait_value:
                        n_dropped += 1
                        continue
                    kept.append(w)
                    ui = updater_idx(w.id, w.wait_value)
                    if ui is not None and ui < idx:
                        for s, v in completion_holds(ui).items():
                            if base.get(s, 0) < v:
                                base[s] = v
                    if base.get(w.id, 0) < w.wait_value:
                        base[w.id] = w.wait_value
                if len(kept) != len(si.on_wait):
                    si.on_wait = kept
            holds_at[idx] = base
            last_eng[e] = idx
_compile_patched = False


def _patch_compile_bir():
    """This walrus build accepts at most ONE sync wait per instruction in
    several encodings (S3_LW matmuls, CTRL NoOp/Drain). Tile legitimately
    emits 2 waits on some instructions, so rewrite the serialized BIR just
    before walrus: keep one wait on the instruction and hoist the rest onto
    same-engine NoOps inserted immediately before it (same dispatch point,
    so semantics are unchanged)."""
    global _compile_patched
    if _compile_patched:
        return
    _compile_patched = True
    import orjson

    from concourse import bass2jax, bass_utils

    orig = bass_utils.compile_bir_kernel

    def _split_waits(bir_json: bytes) -> bytes:
        d = orjson.loads(bir_json)
        changed = False
        for fn in d.get("functions", []):
            for blk in fn.get("blocks", []):
                insts = blk.get("instructions", [])
                out = []
                for inst in insts:
                    si = inst.get("sync_info") or {}
                    ow = si.get("on_wait") or []
                    if len(ow) > 1:
                        changed = True
                        for k, w in enumerate(ow[:-1]):
                            out.append({
                                "debug": inst.get("debug", 0),
                                "engine": inst["engine"],
                                "ins": [],
                                "name": f"{inst['name']}-w{k}",
                                "opcode": "NoOp",
                                "outs": [],
                                "sync_info": {"on_update": [],
                                              "on_wait": [w]},
                            })
                        si["on_wait"] = [ow[-1]]
                    out.append(inst)
                blk["instructions"] = out
        return orjson.dumps(d) if changed else bir_json

    def wrapper(bir_json, tmpdir, neff_name="file.neff"):
        return orig(_split_waits(bir_json), tmpdir, neff_name=neff_name)

    bass_utils.compile_bir_kernel = wrapper
    bass2jax.compile_bir_kernel = wrapper


AF = mybir.ActivationFunctionType
ALU = mybir.AluOpType

B, C, K, H, W_DIM = 8, 256, 64, 128, 128
HW = H * W_DIM  # 16384 pixels per sample
N_CORES = 8
NCH = HW // 128       # 128 pixel chunks of 128
P1_G = 8              # pass-1 DMA groups (16 chunks each)
CPG = NCH // P1_G     # chunks per group = 16


def build_nc():
    _patch_tile_drain()
    _patch_compile_bir()
    nc = bacc.Bacc("TRN2", target_bir_lowering=False, debug=False)
    BF16 = mybir.dt.bfloat16
    F8 = mybir.dt.float8e4
    out_d = nc.dram_tensor("out", (128, 2, HW), BF16, kind="ExternalOutput")
    ins = dict(
        x8t=nc.dram_tensor("x8t", (128, NCH * C), F8, kind="ExternalInput").ap(),
        idxpm=nc.dram_tensor("idxpm", (128, NCH), F32, kind="ExternalInput").ap(),
        oh2=nc.dram_tensor("oh2", (K, HW), F8, kind="ExternalInput").ap(),
        xh=nc.dram_tensor("xh", (C, HW), BF16, kind="ExternalInput").ap(),
        recip=nc.dram_tensor("recip", (K, 1), F32, kind="ExternalInput").ap(),
        wt=nc.dram_tensor("wt", (128, 2 * C), BF16, kind="ExternalInput").ap(),
        cwth=nc.dram_tensor("cwth", (128, 2 * C), BF16, kind="ExternalInput").ap(),
    )

    with tile.TileContext(nc) as tc:
        _body(tc, ins, out_d.ap())
    nc.compile()
    return nc


def _body(tc, ins, out_v):
    nc = tc.nc
    BF16 = mybir.dt.bfloat16
    F8 = mybir.dt.float8e4

    with (
        tc.tile_pool(name="consts", bufs=1) as consts,
        tc.tile_pool(name="xres", bufs=P1_G) as xres,
        tc.tile_pool(name="mid_sb", bufs=1) as mid_sb,
    ):
        # ---- constants / parameter tiles (DMAs are emitted later, after the
        # pass-1 critical feeds: transfers complete in emission order) ----
        ident = consts.tile([128, 128], F32, tag="ident")
        make_identity(nc, ident[:])

        wt_sb = consts.tile([128, 2, C], BF16, tag="wt_sb")    # [e, j, c] = W^T[j*128+e, c]
        cwth_sb = consts.tile([128, 2, C], BF16, tag="cwth_sb")
        recip_sb = consts.tile([K, 1], F32, tag="recip_sb")
        F8 = mybir.dt.float8e4
        oh2_all = consts.tile([K, HW], F8, tag="oh2_all")      # [k, px] one-hot
        idxpm_sb = consts.tile([128, NCH], F32, tag="idxpm_sb")  # [p,a]=idx[a*128+p]
        iota_row = consts.tile([128, K], F32, tag="iota_row")    # [p,k]=k
        iota_row_i = consts.tile([128, K], mybir.dt.int32, tag="iota_row_i")
        nc.gpsimd.iota(iota_row_i[:], pattern=[[1, K]], base=0,
                       channel_multiplier=0)
        nc.vector.tensor_copy(iota_row[:], iota_row_i[:])

        M_sb = mid_sb.tile([128, 2, C], BF16, tag="M_sb")      # M = W @ W^T (symmetric)
        means = mid_sb.tile([K, C], F32, tag="means")
        meansT = mid_sb.tile([128, 2, K], F32, tag="meansT")
        meansT_h = mid_sb.tile([128, 2, K], BF16, tag="meansT_h")
        Q_sb = mid_sb.tile([128, 2, K], F32, tag="Q_sb")
        aggT_h = mid_sb.tile([128, 2, K], BF16, tag="aggT_h")
        B_sb = mid_sb.tile([K, K], F32, tag="B_sb")
        tmp64 = mid_sb.tile([K, K], F32, tag="tmp64")
        negI = consts.tile([K, K], F32, tag="negI")            # -identity(64)
        nc.gpsimd.memset(negI[:], 0.0)
        nc.gpsimd.affine_select(
            out=negI[:], in_=negI[:], compare_op=ALU.not_equal,
            fill=-1.0, base=0, pattern=[[-1, K]], channel_multiplier=1,
        )
        scratch64 = mid_sb.tile([K, 1], F32, tag="scratch64")
        neg_g = mid_sb.tile([K, 1], F32, tag="neg_g")
        e_col = mid_sb.tile([K, 1], F32, tag="e_col")
        tableM = mid_sb.tile([K, C], F32, tag="tableM")
        table2T = mid_sb.tile([K, C], F32, tag="table2T")
        tab8 = mid_sb.tile([K, C], F8, tag="tab8")

        x_tiles = []

        with (
            tc.tile_pool(name="p1_sb", bufs=1) as p1_sb,
            tc.tile_pool(name="psum_sums", bufs=1, space="PSUM") as pp_sums,
            tc.tile_pool(name="psum_mid", bufs=2, space="PSUM") as pp_mid,
            tc.tile_pool(name="psum_fill", bufs=1, space="PSUM") as pp_fill,
        ):
            # pixel-major fp8 x (with c contiguous per chunk) and one-hot.
            # Transfers are partition-split so each logical load spreads over
            # several DMA queues (per-queue streaming is the bottleneck).
            x8t_sb = p1_sb.tile([128, P1_G, CPG, C], F8, tag="x8t_sb")
            oh1_sb = p1_sb.tile([128, P1_G, CPG, K], F8, tag="oh1_sb")
            x8_r = ins["x8t"].rearrange("p (g a c) -> p g a c", g=P1_G, a=CPG)
            # Transfers complete in emission order across the shared DMA
            # fabric, so order = priority: the index map (64KB, feeds the
            # on-chip one-hot builds) and pass-1's first x feed, then small
            # params, then the rest of pass 1, then pass-2 data.
            nc.sync.dma_start(out=idxpm_sb[:], in_=ins["idxpm"])
            nc.sync.dma_start(out=x8t_sb[:, 0, :, :], in_=x8_r[:, 0, :, :])
            for g in range(1, P1_G):
                nc.sync.dma_start(out=x8t_sb[:, g, :, :], in_=x8_r[:, g, :, :])
                if g == 2:
                    nc.sync.dma_start(
                        out=wt_sb[:],
                        in_=ins["wt"].rearrange("p (j c) -> p j c", j=2))
                    nc.sync.dma_start(
                        out=cwth_sb[:],
                        in_=ins["cwth"].rearrange("p (j c) -> p j c", j=2))
            nc.sync.dma_start(out=recip_sb[:], in_=ins["recip"])

            # one-hot lhsT built on the (otherwise idle) vector engine from
            # the 64KB index map: oh1[p, a, k] = (idx[a*128+p] == k), exact
            # in fp8. Built two groups ahead of the matmul consumer.
            def build_oh1(g):
                for a in range(CPG):
                    ch = g * CPG + a
                    nc.vector.tensor_scalar(
                        out=oh1_sb[:, g, a, :], in0=iota_row[:],
                        scalar1=idxpm_sb[:, ch:ch + 1], scalar2=None,
                        op0=ALU.is_equal)

            build_oh1(0)
            build_oh1(1)

            # two accumulator banks (even/odd chunks) so consecutive segment
            # matmuls alternate PSUM banks instead of serializing fill/drain
            psum_sums = pp_sums.tile([K, C], F32, tag="psum_sums")
            psum_sums2 = pp_sums.tile([K, C], F32, tag="psum_sums2")
            sums_f = mid_sb.tile([K, C], F32, tag="sums_f")
            sums_t = mid_sb.tile([K, C], F32, tag="sums_t")

            # HAM keep-alive: the PE clock gate (PE_HAM) demotes to K=4/8
            # when non-transpose matmul activity dips, and once pass 2 runs
            # at the gated clock it never re-promotes. These dep-free filler
            # matmuls into a scratch bank keep measured activity high across
            # the (otherwise PE-idle) middle stage.
            fscr = pp_fill.tile([128, 512], F32, tag="fscr")
            fill_src = []

            def filler(n):
                if not fill_src:
                    return
                xt0 = fill_src[0]
                for _ in range(n):
                    nc.tensor.matmul(
                        fscr[:], cwth_sb[:, 0, 0:128], xt0[:, 0, 0:512],
                        start=True, stop=True)

            # Warm-up: make PE observe the POOL-produced identity, then spin
            # dep-free matmuls so the HAM clock-gate promotes to full rate
            # BEFORE the first pass-1 data lands (the promote needs ~3.4us of
            # sustained activity; without this, early pass-1 runs at 1.2 GHz).
            warm = pp_mid.tile([128, C], F32, tag="pm")
            nc.tensor.transpose(warm[:, 0:128], ident[:], ident[:])
            for _ in range(16):
                nc.tensor.matmul(fscr[:, 0:128], ident[:], ident[:],
                                 start=True, stop=True)

            # preload the Exp activation table so the middle stage does not
            # pay the ~1.3us table load on its critical path
            nc.scalar.activation(scratch64[:], neg_g[:], AF.Exp)

            # ---- pass 1: segment sums over all pixels ----
            TPX = HW // P1_G
            for g in range(P1_G):
                # resident x for pass 2: queued per-group so these DMAs stay
                # behind the pass-1 feeds of later groups
                xt_h = xres.tile([128, 2, TPX], BF16, tag="xres")
                x_tiles.append(xt_h)
                if not fill_src:
                    fill_src.append(xt_h)
                for j in range(2):
                    nc.sync.dma_start(
                        out=xt_h[:, j, :],
                        in_=ins["xh"][j * 128:(j + 1) * 128,
                                      g * TPX:(g + 1) * TPX])
                if g == 1:
                    # gather one-hot rides behind the first two conv tiles:
                    # not needed until the first pass-2 flush, and keeping it
                    # out of pass-1's window lands x8t's last group sooner
                    nc.sync.dma_start(out=oh2_all[:], in_=ins["oh2"])
                if g == 2:
                    # M = W @ W^T, emitted here so its wait on the param DMA
                    # does not delay the first segment matmuls (PE runs its
                    # queue in order); contract e, lhsT/rhs both W^T.
                    for h in range(2):
                        pm = pp_mid.tile([128, C], F32, tag="pm")
                        for j in range(2):
                            nc.tensor.matmul(
                                pm[:], wt_sb[:, j, h * 128:(h + 1) * 128],
                                wt_sb[:, j, :], start=(j == 0), stop=(j == 1),
                            )
                        nc.scalar.copy(M_sb[:, h, :], pm[:])
                if g + 2 < P1_G:
                    build_oh1(g + 2)
                # dep-free filler micro-bursts keep the PE duty cycle above
                # the HAM demotion threshold through the DMA-paced tail
                for a in range(CPG):
                    ch = g * CPG + a
                    acc = psum_sums if a % 2 == 0 else psum_sums2
                    if g >= P1_G - 3 and a % 4 == 0 and fill_src:
                        filler(2)
                    nc.tensor.matmul(
                        acc[:], oh1_sb[:, g, a, :], x8t_sb[:, g, a, :],
                        start=(ch < 2), stop=(ch >= NCH - 2))

            # ---- middle: means -> adjacency -> table ----
            nc.scalar.copy(sums_f[:], psum_sums2[:])
            nc.vector.tensor_add(sums_t[:], psum_sums[:], sums_f[:])
            nc.vector.tensor_scalar(
                out=means[:], in0=sums_t[:], scalar1=recip_sb[:],
                scalar2=None, op0=ALU.mult,
            )
            filler(5)

            # meansT (c on partitions)
            for h in range(2):
                pm = pp_mid.tile([128, C], F32, tag="pm")
                nc.tensor.transpose(
                    pm[:, 0:K], means[:, h * 128:(h + 1) * 128], ident[0:K, 0:K],
                )
                nc.scalar.copy(meansT[:, h, :], pm[:, 0:K])
                filler(4)
            nc.vector.tensor_copy(meansT_h[:], meansT[:])

            # Q = M @ means^T  (use symmetry of M for lhsT slicing; bf16)
            for h in range(2):
                pq = pp_mid.tile([128, C], F32, tag="pm")
                for dj in range(2):
                    nc.tensor.matmul(
                        pq[:, 0:K], M_sb[:, dj, h * 128:(h + 1) * 128],
                        meansT_h[:, dj, :], start=(dj == 0), stop=(dj == 1),
                    )
                nc.scalar.copy(Q_sb[:, h, :], pq[:, 0:K])
                filler(3)

            # G = means @ Q  (64x64, symmetric)
            pg = pp_mid.tile([128, C], F32, tag="pm")
            for h in range(2):
                nc.tensor.matmul(
                    pg[0:K, 0:K], meansT[:, h, :], Q_sb[:, h, :],
                    start=(h == 0), stop=(h == 1),
                )
            filler(4)

            # -g = rowsum(G * (-I));  e_col = exp(-g);  B = exp(2G - g_i)
            nc.vector.scalar_tensor_tensor(
                out=tmp64[:], in0=pg[0:K, 0:K], scalar=1.0, in1=negI[:],
                op0=ALU.mult, op1=ALU.mult, accum_out=neg_g[:],
            )
            nc.scalar.activation(e_col[:], neg_g[:], AF.Exp)
            nc.scalar.activation(B_sb[:], pg[0:K, 0:K], AF.Exp, bias=neg_g[:], scale=2.0)
            filler(4)

            # aggT_raw[c,i] = sum_j B[j,i] means[j,c]
            # (B[j,i] = exp(2G_ij - g_j) already carries e^{-g_j})
            for h in range(2):
                pa = pp_mid.tile([128, C], F32, tag="pm")
                nc.tensor.matmul(
                    pa[:, 0:K], means[:, h * 128:(h + 1) * 128], B_sb[:],
                    start=True, stop=True,
                )
                nc.vector.tensor_copy(aggT_h[:, h, :], pa[:, 0:K])
                filler(2)
            # table2T[k, c_out] = e^{-g_k}*(aggT_raw^T@cwt)[k,:] - means@cwt
            # (bf16 weights: same quantized conv_w as pass 2 uses)
            pt2 = pp_mid.tile([128, C], F32, tag="pm")
            ptm = pp_mid.tile([128, C], F32, tag="pm")
            for j in range(2):
                nc.tensor.matmul(
                    pt2[0:K, :], aggT_h[:, j, :], cwth_sb[:, j, :],
                    start=(j == 0), stop=(j == 1),
                )
            for j in range(2):
                nc.tensor.matmul(
                    ptm[0:K, :], meansT_h[:, j, :], cwth_sb[:, j, :],
                    start=(j == 0), stop=(j == 1),
                )
            nc.scalar.copy(tableM[:], ptm[0:K, :])
            filler(4)
            nc.vector.scalar_tensor_tensor(
                out=table2T[:], in0=pt2[0:K, :], scalar=e_col[:], in1=tableM[:],
                op0=ALU.mult, op1=ALU.subtract,
            )
            # fp8 table for the gather matmul (fp8 x fp8 with the one-hot;
            # |table| << 240 so e4m3 range is safe, ~3% quantization on the
            # gathered term only)
            nc.vector.tensor_copy(tab8[:], table2T[:])
            filler(5)

        # ---- pass 2: out = conv_w @ x + table[index] ----
        # Weight-grouped: per 1024-px group each lhsT serves two back-to-back
        # 512-col matmuls, halving LDWEIGHTS pressure vs per-512 tiles.
        # Output staged per 2048 px so DMA descriptors are 4 KB.
        OST = 2048                             # supergroup / staging chunk
        n_sg = HW // OST                       # 8
        tile_px = HW // P1_G                   # x-tile size
        with (
            tc.tile_pool(name="psum_p2", bufs=1, space="PSUM") as pp2,
            tc.tile_pool(name="p2_sb", bufs=5) as p2_sb,
        ):
            # 2048-px supergroups: each cwth/tab8 weight load serves four
            # back-to-back 512-col matmuls (LDWEIGHTS halved vs 1024-px
            # groups). The gather+evacuation of each (sg, half) is deferred
            # behind the next half's conv matmuls, so the first gather trails
            # the middle stage's tab8 by ~1.7us of conv work, and trailing
            # LDWEIGHTS overlap the conv streams.
            ot_tiles = {}
            pend = []

            def flush_one():
                sg, h, po = pend.pop(0)
                hs = slice(h * 128, (h + 1) * 128)
                for cc in range(4):
                    nc.tensor.matmul(
                        po[:, cc * 512:(cc + 1) * 512], tab8[:, hs],
                        oh2_all[:, sg * OST + cc * 512:
                                sg * OST + (cc + 1) * 512],
                        start=False, stop=True)
                if sg not in ot_tiles:
                    ot_tiles[sg] = p2_sb.tile(
                        [128, 2, OST], BF16, tag="ot", name=f"ot{sg}")
                ot = ot_tiles[sg]
                # PSUM evacuation on ACT only: ScalarE reads PSUM at
                # ~0.85 ns/elem vs DVE's ~3.5 ns/elem for f32 reads. The
                # final supergroup evacuates in halves so the last output
                # DMA starts ~1.7us earlier (shorter end-of-kernel drain).
                if sg == n_sg - 1:
                    for q in range(2):
                        qs = slice(q * (OST // 2), (q + 1) * (OST // 2))
                        nc.scalar.copy(ot[:, h, qs], po[:, qs])
                        nc.sync.dma_start(
                            out=out_v[:, h,
                                      sg * OST + q * (OST // 2):
                                      sg * OST + (q + 1) * (OST // 2)],
                            in_=ot[:, h, qs])
                else:
                    nc.scalar.copy(ot[:, h, :], po[:])
                    nc.sync.dma_start(
                        out=out_v[:, h, sg * OST:(sg + 1) * OST],
                        in_=ot[:, h, :])

            # Only the first supergroup's gather is deferred (to let the
            # middle stage's tab8 land behind ~3us of conv work); afterwards
            # each (sg, h) flushes right after its convs so the ACT
            # evacuation completes before its PSUM banks are re-allocated.
            for sg in range(n_sg):
                ti = (sg * OST) // tile_px
                off = (sg * OST) % tile_px
                xt_h = x_tiles[ti]
                for h in range(2):
                    hs = slice(h * 128, (h + 1) * 128)
                    po = pp2.tile([128, OST], F32, tag=f"po{h}")
                    for j in range(2):
                        for cc in range(4):
                            nc.tensor.matmul(
                                po[:, cc * 512:(cc + 1) * 512],
                                cwth_sb[:, j, hs],
                                xt_h[:, j,
                                     off + cc * 512:off + (cc + 1) * 512],
                                start=(j == 0), stop=False)
                    pend.append((sg, h, po))
                    if sg > 0 or h == 1:
                        while pend:
                            flush_one()
            while pend:
                flush_one()


def _ensure_ntff_hook():
    """Register the axon NTFF profiling hook if the image's antenv lacks it."""
    try:
        from antenv.axon_hooks import get_axon_ntff_profile_hook  # noqa: F401
        return
    except ImportError:
        pass
    import types

    import antenv

    mod = types.ModuleType("antenv.axon_hooks")
    _hook = [None]
    mod.set_axon_ntff_profile_hook = lambda h: _hook.__setitem__(0, h)
    mod.get_axon_ntff_profile_hook = lambda: _hook[0]
    sys.modules["antenv.axon_hooks"] = mod
    antenv.axon_hooks = mod
    try:
        from trn_agent_boot.trn_boot import _ntff_profile_via_ctypes

        so = "/opt/axon/libaxon_pjrt.so"
        if os.path.exists(so):
            mod.set_axon_ntff_profile_hook(_ntff_profile_via_ctypes(so))
    except Exception:
        pass


_NC_CACHE = None
LAST_RESULT = None


def _get_nc():
    global _NC_CACHE
    if _NC_CACHE is None:
        _NC_CACHE = build_nc()
    return _NC_CACHE


def kernel(x, index, W, conv_w):
    """Full inputs in, full output out. Shards batch across 8 NeuronCores."""
    global LAST_RESULT
    from concourse.bass_utils import run_bass_kernel_spmd

    import ml_dtypes

    F8NP = ml_dtypes.float8_e4m3
    x = np.asarray(x, dtype=np.float32).reshape(B, C, HW)
    idx_i = np.asarray(index).reshape(B, HW)
    wt = np.asarray(W, dtype=np.float32).T           # (e, c) = W[c, e]
    cwt = np.asarray(conv_w, dtype=np.float32).reshape(C, C).T

    nc = _get_nc()
    xh = x.astype(ml_dtypes.bfloat16)
    # params pre-arranged to the SBUF layout: [p, j*C + c] = m[j*128+p, c]
    wt_dev = np.ascontiguousarray(
        wt.astype(ml_dtypes.bfloat16).reshape(2, 128, C).transpose(1, 0, 2)
    ).reshape(128, 2 * C)
    cwth_dev = np.ascontiguousarray(
        cwt.astype(ml_dtypes.bfloat16).reshape(2, 128, C).transpose(1, 0, 2)
    ).reshape(128, 2 * C)
    # pixel-major layouts for pass 1, partition-contiguous in DRAM:
    #   x8t[b, p, a*C + c] = x[b, c, a*128 + p]   (fp8 e4m3, clipped)
    #   idxpm[b, p, a] = index[b, a*128 + p]      (feeds on-chip one-hot)
    # and the segment-major one-hot for the pass-2 gather:
    #   oh2[b, k, px] = (index[b, px] == k)
    x8t_all = np.clip(x.reshape(B, C, NCH, 128).transpose(0, 3, 2, 1),
                      -240.0, 240.0).astype(F8NP).reshape(B, 128, NCH * C)
    idx_pm = idx_i.reshape(B, NCH, 128).transpose(0, 2, 1).astype(np.float32)
    oh2_all = (idx_i[:, None, :] == np.arange(K)[None, :, None]).astype(F8NP)
    counts = np.stack([np.bincount(idx_i[b], minlength=K) for b in range(B)])
    recip = (1.0 / np.maximum(counts, 1)).astype(np.float32)[..., None]

    in_maps = [
        {"x8t": np.ascontiguousarray(x8t_all[b]),
         "idxpm": np.ascontiguousarray(idx_pm[b]),
         "oh2": np.ascontiguousarray(oh2_all[b]),
         "xh": np.ascontiguousarray(xh[b]),
         "recip": np.ascontiguousarray(recip[b]),
         "wt": wt_dev, "cwth": cwth_dev}
        for b in range(B)
    ]
    trace = bool(int(os.environ.get("KERNEL_TRACE", "0")))
    if trace:
        try:
            _ensure_ntff_hook()
            res = run_bass_kernel_spmd(
                nc, in_maps, core_ids=list(range(N_CORES)), trace=True,
            )
        except Exception as e:  # profiling must never break the answer path
            print(f"kernel: trace run failed ({e!r}); rerunning untraced")
            res = run_bass_kernel_spmd(
                nc, in_maps, core_ids=list(range(N_CORES)), trace=False,
            )
    else:
        res = run_bass_kernel_spmd(
            nc, in_maps, core_ids=list(range(N_CORES)), trace=False,
        )
    LAST_RESULT = res
    out = np.stack([
        np.asarray(res.results[b]["out"]).astype(np.float32)
        .reshape(128, 2, HW).transpose(1, 0, 2).reshape(C, H, W_DIM)
        for b in range(B)
    ])
    return out
